# revision 9
# baseline (speedup 1.0000x reference)
"""Trainium2 Bass kernel for leave-one-out Nadaraya-Watson regression
(nn_Net_41420664602632, retrieval_knn).

Math
----
reference:
    Fx = x @ W.T ; Ft = train_X @ W.T          [N, 3]
    K[j,i,c] = exp(-((Ft[j,c]-Fx[i,c])/h)^2/2), K[i,i,c] = 0
    out[i,c] = sum_j K[j,i,c]*Y[j,c] / sum_j K[j,i,c]

With a = Ft/(sqrt(2)*h), b = Fx/(sqrt(2)*h):
    K[j,i] = exp(-(a_j-b_i)^2) = exp(-b_i^2) * g[j,i],
    g[j,i] = exp(2*a_j*b_i - a_j^2)
The exp(-b_i^2) factor is common to numerator and denominator and cancels
in the ratio, so the device only computes g and its two j-reductions.

Device program (per core, j-shard of 512 training points)
---------------------------------------------------------
for jt in 4 (j-tiles of 128), c in 3:
    g = ScalarE.activation(Exp, in=bcast(b[:,c]) [128,4096],
                           scale=2*a_j (per-partition), bias=-a_j^2)
    for ic in 8: PE matmul [Y_j,1]^T @ g[:, ic*512:...] -> PSUM[32c:32c+2]
        (fp32, col-tiled at partition offsets 0/32/64, accumulated over jt)
Host sums the 8 cores' [3,2,4096] partials, subtracts the j==i self term,
and divides.
"""

import os

import numpy as np

import concourse.bass as bass
import concourse.tile as tile
from concourse import bacc, mybir
from concourse.bass_utils import run_bass_kernel_spmd

N = 4096       # training/query points
C = 3          # projected channels (fc1 out_features)
NCORES = 8
JSH = N // NCORES        # 512: j-shard per core
JTILES = JSH // 128      # 4
ICH = 512                # moving free-dim chunk = one PSUM bank
NIC = N // ICH           # 8

# bb materialization: "dma" = broadcast DMA from DRAM row (stride-0 source),
# "gpsimd" = DMA row to SBUF then GpSimd partition_broadcast.
BB_MODE = os.environ.get("BB_MODE", "dma")

_CACHE = {}


def _build_nc(n=N, ncores=NCORES, bb_mode=BB_MODE):
    key = (n, ncores, bb_mode)
    if key in _CACHE:
        return _CACHE[key]
    jsh = n // ncores
    jtiles = jsh // 128
    nic = n // ICH
    f32 = mybir.dt.float32
    ncols = C * jtiles * 2
    # stationary is [Y_j, 1, 0...0] padded to 32 columns so the matmul
    # initializes its whole 32-partition PSUM col-group (M=32)
    nstat = C * jtiles * 32

    nc = bacc.Bacc("TRN2", target_bir_lowering=False, debug=False)
    bsrc = nc.dram_tensor("bsrc", [C, n], f32, kind="ExternalInput")
    sb_d = nc.dram_tensor("scalebias", [128, ncols], f32, kind="ExternalInput")
    st_d = nc.dram_tensor("stat", [128, nstat], f32, kind="ExternalInput")
    out_d = nc.dram_tensor("out", [C, 2, n], f32, kind="ExternalOutput")

    with tile.TileContext(nc) as tc:
        with (
            tc.tile_pool(name="const", bufs=1) as constp,
            tc.tile_pool(name="bb", bufs=1) as bbp,
            tc.tile_pool(name="g", bufs=4) as gp,
            tc.tile_pool(name="outsb", bufs=1) as outp,
            tc.tile_pool(name="psum", bufs=1, space=bass.MemorySpace.PSUM) as pp,
        ):
            sb = constp.tile([128, ncols], f32, tag="sb")
            st = constp.tile([128, nstat], f32, tag="st")
            nc.sync.dma_start(sb[:], sb_d.ap())
            nc.sync.dma_start(st[:], st_d.ap())

            bbs = []
            for c in range(C):
                bb = bbp.tile([128, n], f32, tag=f"bb{c}")
                if bb_mode == "dma":
                    nc.sync.dma_start(
                        bb[:], bsrc.ap()[c : c + 1, :].broadcast_to([128, n])
                    )
                else:
                    row = bbp.tile([1, n], f32, tag=f"bbrow{c}")
                    nc.sync.dma_start(row[:], bsrc.ap()[c : c + 1, :])
                    nc.gpsimd.partition_broadcast(bb[:], row[:])
                bbs.append(bb)

            acc = pp.tile([128, n], f32, tag="acc")

            for jt in range(jtiles):
                gs = []
                for c in range(C):
                    g = gp.tile([128, n], f32, tag="g")
                    k = (c * jtiles + jt) * 2
                    nc.scalar.activation(
                        g[:],
                        bbs[c][:],
                        mybir.ActivationFunctionType.Exp,
                        bias=sb[:, k + 1 : k + 2],
                        scale=sb[:, k : k + 1],
                    )
                    gs.append(g)
                for ic in range(nic):
                    for c in range(C):
                        ks = (c * jtiles + jt) * 32
                        nc.tensor.matmul(
                            acc[32 * c : 32 * (c + 1), ic * ICH : (ic + 1) * ICH],
                            lhsT=st[:, ks : ks + 32],
                            rhs=gs[c][:, ic * ICH : (ic + 1) * ICH],
                            start=(jt == 0),
                            stop=(jt == jtiles - 1),
                            tile_position=(0, 32 * c),
                        )

            outsb = outp.tile([128, n], f32, tag="outsb")
            nc.vector.tensor_copy(outsb[0 : 32 * C, :], acc[0 : 32 * C, :])
            for c in range(C):
                nc.sync.dma_start(out_d.ap()[c], outsb[32 * c : 32 * c + 2, :])

    nc.compile()
    _CACHE[key] = nc
    return nc


def _prep_inputs(x, train_X, Y, W, h, n=N, ncores=NCORES):
    """Host-side prep: projections + per-core input maps (all float32)."""
    jsh = n // ncores
    jtiles = jsh // 128
    ncols = C * jtiles * 2
    nstat = C * jtiles * 32
    x64 = np.asarray(x, np.float64)
    t64 = np.asarray(train_X, np.float64)
    W64 = np.asarray(W, np.float64)
    hv = float(np.asarray(h).reshape(-1)[0])
    s = 1.0 / (np.sqrt(2.0) * hv)
    b = (x64 @ W64.T) * s          # queries   [n, C]
    a = (t64 @ W64.T) * s          # training  [n, C]
    a32 = a.astype(np.float32).astype(np.float64)  # device sees fp32 values
    b32 = b.astype(np.float32).astype(np.float64)

    Yf = np.asarray(Y, np.float64)
    bsrc = np.ascontiguousarray(b32.T.astype(np.float32))  # [C, n]

    in_maps = []
    for r in range(ncores):
        j0 = r * jsh
        sbm = np.empty((128, ncols), np.float32)
        stm = np.zeros((128, nstat), np.float32)
        for c in range(C):
            for jt in range(jtiles):
                k = (c * jtiles + jt) * 2
                ks = (c * jtiles + jt) * 32
                aj = a32[j0 + jt * 128 : j0 + (jt + 1) * 128, c]
                sbm[:, k] = (2.0 * aj).astype(np.float32)
                sbm[:, k + 1] = (-(aj * aj)).astype(np.float32)
                stm[:, ks] = Yf[j0 + jt * 128 : j0 + (jt + 1) * 128, c].astype(
                    np.float32
                )
                stm[:, ks + 1] = 1.0
        in_maps.append({"bsrc": bsrc, "scalebias": sbm, "stat": stm})
    return in_maps, a32, b32


def _combine(results, Y, a32, b32, n=N):
    """Sum per-core partials, subtract self term, divide. float64 on host."""
    num = np.zeros((n, C), np.float64)
    den = np.zeros((n, C), np.float64)
    for res in results:
        o = np.asarray(res["out"], np.float64)  # [C, 2, n]
        num += o[:, 0, :].T
        den += o[:, 1, :].T
    # leave-one-out: remove the j == i term g_ii = exp(2 a_i b_i - a_i^2)
    g_self = np.exp(
        np.float32(2.0) * a32.astype(np.float32) * b32.astype(np.float32)
        - np.square(a32.astype(np.float32)),
        dtype=np.float32,
    ).astype(np.float64)
    num -= g_self * np.asarray(Y, np.float64)
    den -= g_self
    return (num / den).astype(np.float32)


def kernel(x, train_X, Y, W, h):
    nc = _build_nc()
    in_maps, a32, b32 = _prep_inputs(x, train_X, Y, W, h)
    res = run_bass_kernel_spmd(nc, in_maps, core_ids=list(range(NCORES)))
    return _combine(res.results, Y, a32, b32)


# revision 12
# speedup vs baseline: 1.1731x; 1.1731x over previous
"""Trainium2 Bass kernel for leave-one-out Nadaraya-Watson regression
(nn_Net_41420664602632, retrieval_knn).

Math
----
reference:
    Fx = x @ W.T ; Ft = train_X @ W.T          [N, 3]
    K[j,i,c] = exp(-((Ft[j,c]-Fx[i,c])/h)^2/2), K[i,i,c] = 0
    out[i,c] = sum_j K[j,i,c]*Y[j,c] / sum_j K[j,i,c]

With a = Ft/(sqrt(2)*h), b = Fx/(sqrt(2)*h):
    K[j,i] = exp(-(a_j-b_i)^2) = exp(-b_i^2) * g[j,i],
    g[j,i] = exp(2*a_j*b_i - a_j^2)
The exp(-b_i^2) factor is common to numerator and denominator and cancels
in the ratio, so the device only computes g and its two j-reductions.

Device program (per core, j-shard of 512 training points)
---------------------------------------------------------
for jt in 4 (j-tiles of 128), c in 3:
    g = ScalarE.activation(Exp, in=bcast(b[:,c]) [128,4096],
                           scale=2*a_j (per-partition), bias=-a_j^2)
    for ic in 8: PE matmul [Y_j,1]^T @ g[:, ic*512:...] -> PSUM[32c:32c+2]
        (fp32, col-tiled at partition offsets 0/32/64, accumulated over jt)
Host sums the 8 cores' [3,2,4096] partials, subtracts the j==i self term,
and divides.
"""

import os

import numpy as np

import concourse.bass as bass
import concourse.tile as tile
from concourse import bacc, mybir
from concourse.bass_utils import run_bass_kernel_spmd

N = 4096       # training/query points
C = 3          # projected channels (fc1 out_features)
NCORES = 8
JSH = N // NCORES        # 512: j-shard per core
JTILES = JSH // 128      # 4
ICH = 512                # moving free-dim chunk = one PSUM bank
NIC = N // ICH           # 8

# bb materialization: "dma" = broadcast DMA from DRAM row (stride-0 source),
# "gpsimd" = DMA row to SBUF then GpSimd partition_broadcast.
BB_MODE = os.environ.get("BB_MODE", "dma")

_CACHE = {}


def _build_nc(n=N, ncores=NCORES, bb_mode=BB_MODE):
    key = (n, ncores, bb_mode)
    if key in _CACHE:
        return _CACHE[key]
    jsh = n // ncores
    jtiles = jsh // 128
    nic = n // ICH
    f32 = mybir.dt.float32
    ncols = C * jtiles * 2

    nc = bacc.Bacc("TRN2", target_bir_lowering=False, debug=False)
    bsrc = nc.dram_tensor("bsrc", [C, n], f32, kind="ExternalInput")
    sb_d = nc.dram_tensor("scalebias", [128, ncols], f32, kind="ExternalInput")
    st_d = nc.dram_tensor("stat", [128, ncols], f32, kind="ExternalInput")
    out_d = nc.dram_tensor("out", [C, 2, n], f32, kind="ExternalOutput")

    with tile.TileContext(nc) as tc:
        with (
            tc.tile_pool(name="const", bufs=1) as constp,
            tc.tile_pool(name="bb", bufs=1) as bbp,
            tc.tile_pool(name="g", bufs=4) as gp,
            tc.tile_pool(name="outsb", bufs=1) as outp,
            tc.tile_pool(name="psum", bufs=1, space=bass.MemorySpace.PSUM) as pp,
        ):
            # warm the ACT exp table set immediately (overlaps input DMAs)
            warm = constp.tile([128, 1], f32, tag="warm")
            nc.gpsimd.memset(warm[:], 0.0)
            nc.scalar.activation(warm[:], warm[:], mybir.ActivationFunctionType.Exp)

            sb = constp.tile([128, ncols], f32, tag="sb")
            st = constp.tile([128, ncols], f32, tag="st")
            nc.sync.dma_start(sb[:], sb_d.ap())
            nc.sync.dma_start(st[:], st_d.ap())

            # PSUM accumulator: zero-fill so partitions the matmuls never
            # touch are defined for the tail copies
            acc = pp.tile([128, n], f32, tag="acc")
            nc.vector.memset(acc[:], 0.0)

            bbs = []
            for c in range(C):
                bb = bbp.tile([128, n], f32, tag=f"bb{c}")
                if bb_mode == "dma":
                    # channel 0 first, chunked so its pieces run on several
                    # DMA engines in parallel and unblock the first ACT op
                    nch = 4 if c == 0 else 2
                    w = n // nch
                    for q in range(nch):
                        nc.sync.dma_start(
                            bb[:, q * w : (q + 1) * w],
                            bsrc.ap()[c : c + 1, q * w : (q + 1) * w].broadcast_to(
                                [128, w]
                            ),
                        )
                else:
                    row = bbp.tile([1, n], f32, tag=f"bbrow{c}")
                    nc.sync.dma_start(row[:], bsrc.ap()[c : c + 1, :])
                    nc.gpsimd.partition_broadcast(bb[:], row[:])
                bbs.append(bb)

            outsb = outp.tile([128, n], f32, tag="outsb")

            for jt in range(jtiles):
                gs = []
                for c in range(C):
                    g = gp.tile([128, n], f32, tag="g")
                    k = (c * jtiles + jt) * 2
                    nc.scalar.activation(
                        g[:],
                        bbs[c][:],
                        mybir.ActivationFunctionType.Exp,
                        bias=sb[:, k + 1 : k + 2],
                        scale=sb[:, k : k + 1],
                    )
                    gs.append(g)
                for kk in range(nic):
                    for c in range(C):
                        # stagger banks so the 3 concurrent col-groups never
                        # write the same PSUM bank in the same beat
                        ic = (kk + 3 * c) % nic
                        k = (c * jtiles + jt) * 2
                        nc.tensor.matmul(
                            acc[32 * c : 32 * c + 2, ic * ICH : (ic + 1) * ICH],
                            lhsT=st[:, k : k + 2],
                            rhs=gs[c][:, ic * ICH : (ic + 1) * ICH],
                            start=(jt == 0),
                            stop=(jt == jtiles - 1),
                            tile_position=(0, 32 * c),
                        )
                if jt == jtiles - 1:
                    # evacuate each bank as soon as its accumulation stops,
                    # overlapping the copies with the remaining matmuls
                    for kk in range(nic):
                        ic = (kk + 3 * (C - 1)) % nic
                        nc.vector.tensor_copy(
                            outsb[:, ic * ICH : (ic + 1) * ICH],
                            acc[:, ic * ICH : (ic + 1) * ICH],
                        )
            for c in range(C):
                nc.sync.dma_start(out_d.ap()[c], outsb[32 * c : 32 * c + 2, :])

    nc.compile()
    _CACHE[key] = nc
    return nc


def _prep_inputs(x, train_X, Y, W, h, n=N, ncores=NCORES):
    """Host-side prep: projections + per-core input maps (all float32)."""
    jsh = n // ncores
    jtiles = jsh // 128
    ncols = C * jtiles * 2
    x64 = np.asarray(x, np.float64)
    t64 = np.asarray(train_X, np.float64)
    W64 = np.asarray(W, np.float64)
    hv = float(np.asarray(h).reshape(-1)[0])
    s = 1.0 / (np.sqrt(2.0) * hv)
    b = (x64 @ W64.T) * s          # queries   [n, C]
    a = (t64 @ W64.T) * s          # training  [n, C]
    a32 = a.astype(np.float32).astype(np.float64)  # device sees fp32 values
    b32 = b.astype(np.float32).astype(np.float64)

    Yf = np.asarray(Y, np.float64)
    bsrc = np.ascontiguousarray(b32.T.astype(np.float32))  # [C, n]

    in_maps = []
    for r in range(ncores):
        j0 = r * jsh
        sbm = np.empty((128, ncols), np.float32)
        stm = np.zeros((128, ncols), np.float32)
        for c in range(C):
            for jt in range(jtiles):
                k = (c * jtiles + jt) * 2
                aj = a32[j0 + jt * 128 : j0 + (jt + 1) * 128, c]
                sbm[:, k] = (2.0 * aj).astype(np.float32)
                sbm[:, k + 1] = (-(aj * aj)).astype(np.float32)
                stm[:, k] = Yf[j0 + jt * 128 : j0 + (jt + 1) * 128, c].astype(
                    np.float32
                )
                stm[:, k + 1] = 1.0
        in_maps.append({"bsrc": bsrc, "scalebias": sbm, "stat": stm})
    return in_maps, a32, b32


def _combine(results, Y, a32, b32, n=N):
    """Sum per-core partials, subtract self term, divide. float64 on host."""
    num = np.zeros((n, C), np.float64)
    den = np.zeros((n, C), np.float64)
    for res in results:
        o = np.asarray(res["out"], np.float64)  # [C, 2, n]
        num += o[:, 0, :].T
        den += o[:, 1, :].T
    # leave-one-out: remove the j == i term g_ii = exp(2 a_i b_i - a_i^2)
    g_self = np.exp(
        np.float32(2.0) * a32.astype(np.float32) * b32.astype(np.float32)
        - np.square(a32.astype(np.float32)),
        dtype=np.float32,
    ).astype(np.float64)
    num -= g_self * np.asarray(Y, np.float64)
    den -= g_self
    return (num / den).astype(np.float32)


def kernel(x, train_X, Y, W, h):
    nc = _build_nc()
    in_maps, a32, b32 = _prep_inputs(x, train_X, Y, W, h)
    res = run_bass_kernel_spmd(nc, in_maps, core_ids=list(range(NCORES)))
    return _combine(res.results, Y, a32, b32)


# revision 21
# speedup vs baseline: 1.3933x; 1.1877x over previous
"""Trainium2 Bass kernel for leave-one-out Nadaraya-Watson regression
(nn_Net_41420664602632, retrieval_knn).

Math
----
reference:
    Fx = x @ W.T ; Ft = train_X @ W.T          [N, 3]
    K[j,i,c] = exp(-((Ft[j,c]-Fx[i,c])/h)^2/2), K[i,i,c] = 0
    out[i,c] = sum_j K[j,i,c]*Y[j,c] / sum_j K[j,i,c]

With a = Ft/(sqrt(2)*h), b = Fx/(sqrt(2)*h):
    K[j,i] = exp(-(a_j-b_i)^2) = exp(-b_i^2) * g[j,i],
    g[j,i] = exp(2*a_j*b_i - a_j^2)
The exp(-b_i^2) factor is common to numerator and denominator and cancels
in the ratio, so the device only computes g and its two j-reductions.

Device program (per core, j-shard of 512 training points)
---------------------------------------------------------
for jt in 4 (j-tiles of 128), c in 3:
    g = ScalarE.activation(Exp, in=bcast(b[:,c]) [128,4096],
                           scale=2*a_j (per-partition), bias=-a_j^2)
    for ic in 8: PE matmul [Y_j,1]^T @ g[:, ic*512:...] -> PSUM[32c:32c+2]
        (fp32, col-tiled at partition offsets 0/32/64, accumulated over jt)
Host sums the 8 cores' [3,2,4096] partials, subtracts the j==i self term,
and divides.
"""

import os

import numpy as np

import concourse.bass as bass
import concourse.tile as tile
from concourse import bacc, mybir
from concourse.bass_utils import run_bass_kernel_spmd

N = 4096       # training/query points
C = 3          # projected channels (fc1 out_features)
NCORES = 8
JSH = N // NCORES        # 512: j-shard per core
JTILES = JSH // 128      # 4
ICH = 512                # moving free-dim chunk = one PSUM bank
NIC = N // ICH           # 8

# bb materialization: "dma" = broadcast DMA from DRAM row (stride-0 source),
# "gpsimd" = DMA row to SBUF then GpSimd partition_broadcast.
BB_MODE = os.environ.get("BB_MODE", "dma")
# matmul operand dtype: "f32" exact 2-pass half-speed ("f32r" is broken on
# this toolchain: known all-zero HW output for float32r weight loads)
MM_DTYPE = os.environ.get("MM_DTYPE", "f32")

_CACHE = {}


def _build_nc(n=N, ncores=NCORES, bb_mode=BB_MODE, mm_dtype=MM_DTYPE):
    key = (n, ncores, bb_mode, mm_dtype)
    if key in _CACHE:
        return _CACHE[key]
    jsh = n // ncores
    jtiles = jsh // 128
    nic = n // ICH
    f32 = mybir.dt.float32
    ncols = C * jtiles * 2

    nc = bacc.Bacc("TRN2", target_bir_lowering=False, debug=False)
    bsrc = nc.dram_tensor("bsrc", [C, n], f32, kind="ExternalInput")
    sb_d = nc.dram_tensor("scalebias", [128, ncols], f32, kind="ExternalInput")
    st_d = nc.dram_tensor("stat", [128, ncols], f32, kind="ExternalInput")
    out_d = nc.dram_tensor("out", [C, 2, n], f32, kind="ExternalOutput")

    with tile.TileContext(nc) as tc:
        with (
            tc.tile_pool(name="const", bufs=1) as constp,
            tc.tile_pool(name="bb", bufs=1) as bbp,
            tc.tile_pool(name="g", bufs=4) as gp,
            tc.tile_pool(name="outsb", bufs=1) as outp,
            tc.tile_pool(name="psum", bufs=1, space=bass.MemorySpace.PSUM) as pp,
        ):
            # warm the ACT exp table set immediately (overlaps input DMAs)
            warm = constp.tile([128, 1], f32, tag="warm")
            nc.gpsimd.memset(warm[:], 0.0)
            nc.scalar.activation(warm[:], warm[:], mybir.ActivationFunctionType.Exp)

            mmdt = mybir.dt.float32r if mm_dtype == "f32r" else f32
            sb = constp.tile([128, ncols], f32, tag="sb")
            st = constp.tile([128, ncols], mmdt, tag="st")
            nc.sync.dma_start(sb[:], sb_d.ap())
            nc.sync.dma_start(st[:], st_d.ap().bitcast(mmdt))

            acc = pp.tile([128, n], f32, tag="acc")

            bbs = []
            for c in range(C):
                bb = bbp.tile([128, n], f32, tag=f"bb{c}")
                if bb_mode == "dma":
                    # channel 0 first, chunked so its pieces run on several
                    # DMA engines in parallel and unblock the first ACT op
                    nch = 8 if c == 0 else 4
                    w = n // nch
                    for q in range(nch):
                        nc.sync.dma_start(
                            bb[:, q * w : (q + 1) * w],
                            bsrc.ap()[c : c + 1, q * w : (q + 1) * w].broadcast_to(
                                [128, w]
                            ),
                        )
                else:
                    row = bbp.tile([1, n], f32, tag=f"bbrow{c}")
                    nc.sync.dma_start(row[:], bsrc.ap()[c : c + 1, :])
                    nc.gpsimd.partition_broadcast(bb[:], row[:])
                bbs.append(bb)

            # num/den pairs land at outsb[0:2, c*n + ic*ICH : ...]
            outsb = outp.tile([2, C * n], f32, tag="outsb")

            # channel-major: channel 0 computes while later broadcasts land.
            # col-group (c+ic)%4 rotates so consecutive matmuls hit distinct
            # array col-groups AND distinct PSUM banks.
            for c in range(C):
                for jt in range(jtiles):
                    g = gp.tile([128, n], mmdt, tag="g")
                    k = (c * jtiles + jt) * 2
                    nc.scalar.activation(
                        g[:],
                        bbs[c][:],
                        mybir.ActivationFunctionType.Exp,
                        bias=sb[:, k + 1 : k + 2],
                        scale=sb[:, k : k + 1],
                    )
                    for ic in range(nic):
                        grp = 32 * ((c + ic) % 4)
                        nc.tensor.matmul(
                            acc[grp : grp + 2, ic * ICH : (ic + 1) * ICH],
                            lhsT=st[:, k : k + 2],
                            rhs=g[:, ic * ICH : (ic + 1) * ICH],
                            start=(jt == 0),
                            stop=(jt == jtiles - 1),
                            tile_position=(0, grp),
                        )
                # evacuate each slot as its accumulation stops; overlaps the
                # next channel's compute
                for ic in range(nic):
                    grp = 32 * ((c + ic) % 4)
                    nc.vector.tensor_copy(
                        outsb[:, c * n + ic * ICH : c * n + (ic + 1) * ICH],
                        acc[grp : grp + 2, ic * ICH : (ic + 1) * ICH],
                    )
            for c in range(C):
                nc.sync.dma_start(
                    out_d.ap()[c], outsb[:, c * n : (c + 1) * n]
                )

    nc.compile()
    _CACHE[key] = nc
    return nc


def _prep_inputs(x, train_X, Y, W, h, n=N, ncores=NCORES):
    """Host-side prep: projections + per-core input maps (all float32)."""
    jsh = n // ncores
    jtiles = jsh // 128
    ncols = C * jtiles * 2
    x64 = np.asarray(x, np.float64)
    t64 = np.asarray(train_X, np.float64)
    W64 = np.asarray(W, np.float64)
    hv = float(np.asarray(h).reshape(-1)[0])
    s = 1.0 / (np.sqrt(2.0) * hv)
    b = (x64 @ W64.T) * s          # queries   [n, C]
    a = (t64 @ W64.T) * s          # training  [n, C]
    a32 = a.astype(np.float32).astype(np.float64)  # device sees fp32 values
    b32 = b.astype(np.float32).astype(np.float64)

    Yf = np.asarray(Y, np.float64)
    bsrc = np.ascontiguousarray(b32.T.astype(np.float32))  # [C, n]

    in_maps = []
    for r in range(ncores):
        j0 = r * jsh
        sbm = np.empty((128, ncols), np.float32)
        stm = np.zeros((128, ncols), np.float32)
        for c in range(C):
            for jt in range(jtiles):
                k = (c * jtiles + jt) * 2
                aj = a32[j0 + jt * 128 : j0 + (jt + 1) * 128, c]
                sbm[:, k] = (2.0 * aj).astype(np.float32)
                sbm[:, k + 1] = (-(aj * aj)).astype(np.float32)
                stm[:, k] = Yf[j0 + jt * 128 : j0 + (jt + 1) * 128, c].astype(
                    np.float32
                )
                stm[:, k + 1] = 1.0
        in_maps.append({"bsrc": bsrc, "scalebias": sbm, "stat": stm})
    return in_maps, a32, b32


def _combine(results, Y, a32, b32, n=N):
    """Sum per-core partials, subtract self term, divide. float64 on host."""
    num = np.zeros((n, C), np.float64)
    den = np.zeros((n, C), np.float64)
    for res in results:
        o = np.asarray(res["out"], np.float64)  # [C, 2, n]
        num += o[:, 0, :].T
        den += o[:, 1, :].T
    # leave-one-out: remove the j == i term g_ii = exp(2 a_i b_i - a_i^2)
    g_self = np.exp(
        np.float32(2.0) * a32.astype(np.float32) * b32.astype(np.float32)
        - np.square(a32.astype(np.float32)),
        dtype=np.float32,
    ).astype(np.float64)
    num -= g_self * np.asarray(Y, np.float64)
    den -= g_self
    return (num / den).astype(np.float32)


def kernel(x, train_X, Y, W, h):
    nc = _build_nc()
    in_maps, a32, b32 = _prep_inputs(x, train_X, Y, W, h)
    res = run_bass_kernel_spmd(nc, in_maps, core_ids=list(range(NCORES)))
    return _combine(res.results, Y, a32, b32)
